# revision 11
# baseline (speedup 1.0000x reference)
"""Causal self-attention kernel for 8 Trainium2 NeuronCores.

Problem: B=2, T=2048, D=2048, H=16, Dh=128, fp32 in/out.
  qkv = x @ Wqkv + bqkv ; per-head causal attention ; out = att @ Wout + bout

Sharding (tensor parallel over heads + AllToAll before out_proj):
  Core c owns heads {2c, 2c+1}. Each core computes, for all 4096 tokens,
  Q^T/K^T (head-dim on partitions) and V (token-dim on partitions) for its
  two heads via the QKV projection with its 768-column shard of Wqkv, runs
  causal attention locally (scores are computed transposed: S^T[k,q]), and
  produces att^T [256, 2048] per batch. Four small AllToAlls (one per
  half-batch of tokens) redistribute from head-sharded to token-sharded;
  core c then projects its 128-token slices with the full Wout.

All matmul operands are bf16 (inputs cast on host): enables FWL weight
loads, halves HBM/SBUF/collective traffic, full PE stream rate. PSUM
accumulation stays fp32. The V bias is folded into the output bias on host
(softmax rows sum to 1, so +bv propagates as +bv@Wout on every token).
The softmax denominator is accumulated on the vector engine (exp blocks
summed into an fp32 tile) with a single all-ones f32r matmul per group to
reduce over partitions, instead of one ones-matmul per k-block.

Schedule: batch-0 attention interleaves with batch-1's projection; batch-0
out-projection units interleave with batch-1's attention groups. Wout/bout
stream into SBUF during phase 2 (paced, one 512-col chunk per projection
chunk) so the out-projection never waits on HBM. wqkv loads ride the ACT
DMA ring in parallel with x-chunk loads on the sync ring at startup.
"""

import numpy as np
import ml_dtypes

import concourse.bass as bass
import concourse.mybir as mybir
import concourse.tile as tile
from concourse import bacc
from concourse.bass_utils import run_bass_kernel_spmd

B, T, D, H, Dh = 2, 2048, 2048, 16, 128
NT = B * T                  # 4096 tokens total
W = 8                       # cores
HL = H // W                 # 2 heads per core
CQKV = 3 * HL * Dh          # 768 qkv columns per core
KO = D // 128               # 16 contraction subtiles
TC = 512                    # token chunk for projection rhs
NTC_B = T // TC             # 4 chunks per batch
QC = 512                    # attention q-chunk
NQC = T // QC               # 4 q-chunks per batch
HT = T // 2                 # half-batch token span (one AllToAll each)
TOKH = HT // W              # 128 tokens per core per half-batch exchange
SCALE = 1.0 / float(np.sqrt(Dh))

F32 = mybir.dt.float32
F32R = mybir.dt.float32r
BF16 = mybir.dt.bfloat16
NPBF16 = ml_dtypes.bfloat16


def _build():
    nc = bacc.Bacc("TRN2", target_bir_lowering=False, debug=False,
                   enable_asserts=True, num_devices=W)
    xT = nc.dram_tensor("xT", [D, NT], BF16, kind="ExternalInput").ap()
    wqkv = nc.dram_tensor("wqkv", [D, CQKV], BF16, kind="ExternalInput").ap()
    bqk = nc.dram_tensor("bqk", [2 * HL * 128], F32, kind="ExternalInput").ap()
    wout = nc.dram_tensor("wout", [D, D], BF16, kind="ExternalInput").ap()
    masktri = nc.dram_tensor("masktri", [128, 128], BF16, kind="ExternalInput").ap()
    ones = nc.dram_tensor("ones", [128, 128], F32, kind="ExternalInput").ap()
    boutbc = nc.dram_tensor("boutbc", [128, D], F32, kind="ExternalInput").ap()
    # rows [(b*2+half)*TOKH ...): tokens [half*HT + c*TOKH ...) of batch b
    out = nc.dram_tensor("out", [B * 2 * TOKH, D], F32, kind="ExternalOutput").ap()

    xT_v = xT.rearrange("(ko p) t -> p ko t", p=128)
    wqkv_v = wqkv.rearrange("(ko p) c -> p ko c", p=128)
    wout_v = wout.rearrange("(ko p) c -> p ko c", p=128)

    with tile.TileContext(nc) as tc:
        with tc.tile_pool(name="persist", bufs=1) as persist, \
             tc.tile_pool(name="wout_pool", bufs=1) as wout_pool, \
             tc.tile_pool(name="dram", bufs=1, space="DRAM") as dram_pool:
            mask_sb = persist.tile([128, 128], BF16)
            ones_sb = persist.tile([128, 128], F32R)
            bqk_sb = persist.tile([128, 2 * HL], F32)      # Q,K bias (col on partition)
            bout_sb = persist.tile([128, D], F32)
            wout_all = wout_pool.tile([128, KO, D], BF16)

            nc.sync.dma_start(mask_sb[:], masktri)
            nc.sync.dma_start(ones_sb[:], ones.bitcast(F32R))
            nc.sync.dma_start(bqk_sb[:], bqk.rearrange("(cc p) -> p cc", p=128))

            a2a_in = [[dram_pool.tile([W, HL * 128, TOKH], BF16, name=f"a2a_in{b}{h}")
                       for h in range(2)] for b in range(B)]
            a2a_out = [[dram_pool.tile([W, HL * 128, TOKH], BF16, name=f"a2a_out{b}{h}")
                        for h in range(2)] for b in range(B)]

            def alloc_qkv(pool):
                qT = pool.tile([128, HL, T], BF16, name="qT")
                kT = pool.tile([128, HL, T], BF16, name="kT")
                v = pool.tile([128, HL, T // 128, Dh], BF16, name="v")
                return qT, kT, v

            def emit_proj_chunk(qkv, wqkv_sb, x_pool, proj_psum, b, tci):
                """Project one 512-token chunk of batch b into (qT, kT, v)."""
                qT_sb, kT_sb, v_sb = qkv
                t0 = b * T + tci * TC
                x_sb = x_pool.tile([128, KO, TC], BF16, name="x_sb")
                # quartered so the first ko-group's matmuls start early
                for kq in range(4):
                    nc.sync.dma_start(
                        x_sb[:, kq * 4:(kq + 1) * 4, :],
                        xT_v[:, kq * 4:(kq + 1) * 4, t0:t0 + TC])
                for cc in range(2 * HL):
                    ps = proj_psum.tile([128, TC], F32, name="proj_ps")
                    for ko in range(KO):
                        nc.tensor.matmul(
                            ps[:], wqkv_sb[ko][:, cc * 128:(cc + 1) * 128],
                            x_sb[:, ko, :], start=(ko == 0), stop=(ko == KO - 1))
                    dest = qT_sb if cc < HL else kT_sb
                    hl = cc if cc < HL else cc - HL
                    nc.vector.tensor_scalar_add(
                        dest[:, hl, tci * TC:(tci + 1) * TC], ps[:],
                        bqk_sb[:, cc:cc + 1])
                for tb in range(TC // 128):
                    ps = proj_psum.tile([128, TC], F32, name="proj_ps")
                    for ko in range(KO):
                        nc.tensor.matmul(
                            ps[:, :HL * Dh], x_sb[:, ko, tb * 128:(tb + 1) * 128],
                            wqkv_sb[ko][:, 2 * HL * 128:], start=(ko == 0), stop=(ko == KO - 1))
                    idx = tci * (TC // 128) + tb
                    nc.vector.tensor_copy(
                        v_sb[:, :, idx, :],
                        ps[:, :HL * Dh].rearrange("p (hl d) -> p hl d", hl=HL))

            def emit_attn_group(qkv, att_sb, pools, hl, qc):
                """One (head, q-chunk) attention group: S^T -> exp -> P^T V.

                k-blocks are processed in pairs sharing one 2-bank PSUM tile
                so off-diagonal pairs need a single exp over 1024 columns.
                The denominator accumulates on DVE in fp32; one f32r
                ones-matmul per group reduces it over partitions.
                """
                qT_sb, kT_sb, v_sb = qkv
                ex_pool, exsum_pool, rden_pool, s_psum, av_psum, d_psum = pools
                q0 = qc * QC
                nkb = (qc + 1) * (QC // 128)
                ps_av = av_psum.tile([128, QC], F32, name="ps_av")
                ex_sum = exsum_pool.tile([128, QC], F32R, name="ex_sum")
                for kbp in range(nkb // 2):
                    kbs = (2 * kbp, 2 * kbp + 1)
                    os_ = [kb - qc * (QC // 128) for kb in kbs]
                    vss = [max(0, o) * 128 for o in os_]
                    ps_s2 = s_psum.tile([128, 2, QC], F32, name="ps_s2")
                    ex2 = ex_pool.tile([128, 2, QC], BF16, name="ex2")
                    for i, kb in enumerate(kbs):
                        nc.tensor.matmul(
                            ps_s2[:, i, vss[i]:], kT_sb[:, hl, kb * 128:(kb + 1) * 128],
                            qT_sb[:, hl, q0 + vss[i]:q0 + QC], start=True, stop=True)
                    if vss[0] == 0 and vss[1] == 0:
                        nc.scalar.activation(
                            ex2[:], ps_s2[:], mybir.ActivationFunctionType.Exp,
                            scale=SCALE)
                    else:
                        for i in range(2):
                            nc.scalar.activation(
                                ex2[:, i, vss[i]:], ps_s2[:, i, vss[i]:],
                                mybir.ActivationFunctionType.Exp, scale=SCALE)
                    for i, kb in enumerate(kbs):
                        if os_[i] >= 0:
                            nc.vector.tensor_tensor(
                                ex2[:, i, vss[i]:vss[i] + 128],
                                ex2[:, i, vss[i]:vss[i] + 128], mask_sb[:],
                                mybir.AluOpType.mult)
                        nc.tensor.matmul(
                            ps_av[:, vss[i]:], v_sb[:, hl, kb, :], ex2[:, i, vss[i]:],
                            start=(kb == 0), stop=(kb == nkb - 1))
                        if kb == 0:
                            nc.gpsimd.tensor_copy(ex_sum[:], ex2[:, i, :])
                        else:
                            nc.gpsimd.tensor_tensor(
                                ex_sum[:, vss[i]:], ex_sum[:, vss[i]:],
                                ex2[:, i, vss[i]:], mybir.AluOpType.add)
                ps_dbc = d_psum.tile([128, QC], F32, name="ps_dbc")
                nc.tensor.matmul(ps_dbc[:], ones_sb[:], ex_sum[:],
                                 start=True, stop=True)
                rden = rden_pool.tile([128, QC], F32, name="rden")
                nc.vector.reciprocal_approx_fast(rden[:], ps_dbc[:])
                nc.vector.tensor_tensor(
                    att_sb[:, hl, q0:q0 + QC], ps_av[:], rden[:],
                    mybir.AluOpType.mult)

            def emit_stage(att_sb, b, hl, qc):
                """Stage one group's att^T into the A2A input as soon as it
                is normalized, so the collective fires with no staging tail."""
                half = qc // 2
                r0 = (qc % 2) * 4
                nc.gpsimd.dma_start(
                    a2a_in[b][half][r0:r0 + 4, hl * 128:(hl + 1) * 128, :]
                        .rearrange("r p t -> p r t"),
                    att_sb[:, hl, qc * QC:(qc + 1) * QC]
                        .rearrange("p (r t) -> p r t", r=4))

            def emit_a2a(b, half):
                nc.gpsimd.collective_compute(
                    "AllToAll", mybir.AluOpType.bypass,
                    replica_groups=[list(range(W))],
                    ins=[a2a_in[b][half][:].opt()], outs=[a2a_out[b][half][:].opt()])

            def emit_attall_load(attall_pool, b, half):
                attall_sb = attall_pool.tile([128, KO, TOKH], BF16, name="attall")
                nc.sync.dma_start(
                    attall_sb[:],
                    a2a_out[b][half][:].rearrange(
                        "r (x p) t -> p (r x) t", x=HL, p=128))
                return attall_sb

            def emit_outproj_unit(attall_sb, o_pool, out_psum, b, half, colc):
                ps_o = out_psum.tile([128, 512], F32, name="ps_o")
                for ko in range(KO):
                    nc.tensor.matmul(
                        ps_o[:], attall_sb[:, ko, :],
                        wout_all[:, ko, colc * 512:(colc + 1) * 512],
                        start=(ko == 0), stop=(ko == KO - 1))
                o_sb = o_pool.tile([128, 512], F32, name="o_sb")
                nc.vector.tensor_tensor(
                    o_sb[:], ps_o[:],
                    bout_sb[:, colc * 512:(colc + 1) * 512],
                    mybir.AluOpType.add)
                nc.sync.dma_start(
                    out[(b * 2 + half) * TOKH:(b * 2 + half + 1) * TOKH,
                        colc * 512:(colc + 1) * 512],
                    o_sb[:])

            # heavy half (qc 2,3) first so the last A2A covers the small half
            groups_h0 = [(hl, qc) for qc in (1, 0) for hl in range(HL)]
            groups_h1 = [(hl, qc) for qc in (3, 2) for hl in range(HL)]

            with tc.tile_pool(name="qkv1_pool", bufs=1) as qkv1_pool:
                qkv1 = alloc_qkv(qkv1_pool)
                with tc.tile_pool(name="qkv0_pool", bufs=1) as qkv0_pool:
                    qkv0 = alloc_qkv(qkv0_pool)
                    with tc.tile_pool(name="att0_pool", bufs=1) as att0_pool:
                        att0_sb = att0_pool.tile([128, HL, T], BF16)
                        with tc.tile_pool(name="wq_pool", bufs=1) as wq_pool, \
                             tc.tile_pool(name="x_pool", bufs=2) as x_pool, \
                             tc.tile_pool(name="proj_psum", bufs=2, space="PSUM") as proj_psum, \
                             tc.tile_pool(name="ex0_pool", bufs=2) as ex0_pool, \
                             tc.tile_pool(name="exsum0_pool", bufs=2) as exsum0_pool, \
                             tc.tile_pool(name="rden0_pool", bufs=2) as rden0_pool, \
                             tc.tile_pool(name="s0_psum", bufs=2, space="PSUM") as s0_psum, \
                             tc.tile_pool(name="av0_psum", bufs=1, space="PSUM") as av0_psum, \
                             tc.tile_pool(name="d0_psum", bufs=1, space="PSUM") as d0_psum:
                            wqkv_sb = [wq_pool.tile([128, CQKV], BF16,
                                                     name=f"wqkv{ko}", bufs=1)
                                       for ko in range(KO)]
                            # wqkv on the ACT HWDGE ring, in parallel with the
                            # sync-ring x loads, so the PE starts early
                            for ko in range(KO):
                                nc.scalar.dma_start(
                                    wqkv_sb[ko][:],
                                    wqkv_v[:, ko, :])
                            pools0 = (ex0_pool, exsum0_pool, rden0_pool,
                                      s0_psum, av0_psum, d0_psum)
                            # batch-0 projection
                            for tci in range(NTC_B):
                                emit_proj_chunk(qkv0, wqkv_sb, x_pool, proj_psum, 0, tci)
                            # batch-1 projection interleaved with batch-0
                            # attention; wout/bout stream in paced chunks
                            nc.sync.dma_start(bout_sb[:], boutbc)
                            groups0 = groups_h1 + groups_h0
                            for i in range(NTC_B):
                                emit_proj_chunk(qkv1, wqkv_sb, x_pool, proj_psum, 1, i)
                                nc.sync.dma_start(
                                    wout_all[:, :, i * 512:(i + 1) * 512],
                                    wout_v[:, :, i * 512:(i + 1) * 512])
                                for g in (groups0[2 * i], groups0[2 * i + 1]):
                                    emit_attn_group(qkv0, att0_sb, pools0, *g)
                                    emit_stage(att0_sb, 0, *g)
                                if i == NTC_B // 2 - 1:
                                    emit_a2a(0, 1)
                        emit_a2a(0, 0)
                # batch-1 attention interleaved with batch-0 out-proj units;
                # the A2As overlap the following PE work
                with tc.tile_pool(name="att1_pool", bufs=1) as att1_pool:
                    att1_sb = att1_pool.tile([128, HL, T], BF16)
                    with tc.tile_pool(name="ex1_pool", bufs=3) as ex1_pool, \
                         tc.tile_pool(name="exsum1_pool", bufs=2) as exsum1_pool, \
                         tc.tile_pool(name="rden1_pool", bufs=2) as rden1_pool, \
                         tc.tile_pool(name="s1_psum", bufs=2, space="PSUM") as s1_psum, \
                         tc.tile_pool(name="av1_psum", bufs=1, space="PSUM") as av1_psum, \
                         tc.tile_pool(name="d1_psum", bufs=1, space="PSUM") as d1_psum, \
                         tc.tile_pool(name="attall_pool", bufs=4) as attall_pool, \
                         tc.tile_pool(name="o_pool", bufs=3) as o_pool, \
                         tc.tile_pool(name="out_psum", bufs=2, space="PSUM") as out_psum:
                        pools1 = (ex1_pool, exsum1_pool, rden1_pool,
                                  s1_psum, av1_psum, d1_psum)
                        attall01 = emit_attall_load(attall_pool, 0, 1)
                        attall00 = emit_attall_load(attall_pool, 0, 0)
                        # att1 heavy half back-to-back so a2a(1,1) fires as
                        # early as possible (the two batch-1 A2As serialize
                        # on the CC core)
                        for g in groups_h1:
                            emit_attn_group(qkv1, att1_sb, pools1, *g)
                            emit_stage(att1_sb, 1, *g)
                        emit_a2a(1, 1)
                        # batch-0 out-proj units interleave with the light half
                        emit_outproj_unit(attall01, o_pool, out_psum, 0, 1, 0)
                        emit_outproj_unit(attall01, o_pool, out_psum, 0, 1, 1)
                        unit_seq = [(attall01, 0, 1, 2), (attall01, 0, 1, 3),
                                    (attall00, 0, 0, 0), (attall00, 0, 0, 1)]
                        for j, g in enumerate(groups_h0):
                            emit_attn_group(qkv1, att1_sb, pools1, *g)
                            emit_stage(att1_sb, 1, *g)
                            asb, b_, h_, c_ = unit_seq[j]
                            emit_outproj_unit(asb, o_pool, out_psum, b_, h_, c_)
                        emit_a2a(1, 0)
                        emit_outproj_unit(attall00, o_pool, out_psum, 0, 0, 2)
                        emit_outproj_unit(attall00, o_pool, out_psum, 0, 0, 3)
                        attall11 = emit_attall_load(attall_pool, 1, 1)
                        for colc in range(4):
                            emit_outproj_unit(attall11, o_pool, out_psum, 1, 1, colc)
                        attall10 = emit_attall_load(attall_pool, 1, 0)
                        for colc in range(4):
                            emit_outproj_unit(attall10, o_pool, out_psum, 1, 0, colc)
    nc.compile()
    return nc


_CACHED_NC = None


def kernel(x, Wqkv, bqkv, Wout, bout):
    global _CACHED_NC
    x = np.asarray(x, dtype=np.float32)
    Wqkv = np.asarray(Wqkv, dtype=np.float32)
    bqkv = np.asarray(bqkv, dtype=np.float32)
    Wout = np.asarray(Wout, dtype=np.float32)
    bout = np.asarray(bout, dtype=np.float32)

    if _CACHED_NC is None:
        _CACHED_NC = _build()
    nc = _CACHED_NC

    xT = np.ascontiguousarray(x.reshape(NT, D).T).astype(NPBF16)  # [D, NT]
    wq4 = Wqkv.reshape(D, 3, H, Dh)                        # col = which, head, dh
    bq4 = bqkv.reshape(3, H, Dh)
    kl = np.arange(128)[:, None]
    jl = np.arange(128)[None, :]
    masktri = (jl >= kl).astype(NPBF16)
    # softmax rows sum to 1, so the V bias contributes bv @ Wout to every token
    bv_full = bqkv[2 * D:3 * D].astype(np.float64)
    bout_eff = (bout.astype(np.float64) + bv_full @ Wout.astype(np.float64)
                ).astype(np.float32)

    in_maps = []
    for c in range(W):
        wshard = np.ascontiguousarray(
            wq4[:, :, HL * c:HL * c + HL, :].reshape(D, CQKV)).astype(NPBF16)
        bshard = np.ascontiguousarray(
            bq4[:2, HL * c:HL * c + HL, :].reshape(2 * HL * 128))
        in_maps.append({
            "xT": xT, "wqkv": wshard, "bqk": bshard,
            "wout": Wout.astype(NPBF16), "masktri": masktri,
            "ones": np.ones((128, 128), np.float32),
            "boutbc": np.tile(bout_eff[None, :], (128, 1)),
        })

    res = run_bass_kernel_spmd(nc, in_maps, core_ids=list(range(W)))
    # res[c]["out"] rows [(b*2+h)*TOKH ...) = tokens [h*HT + c*TOKH ...) of batch b
    full = np.empty((B, T, D), np.float32)
    for c in range(W):
        for b in range(B):
            for h in range(2):
                full[b, h * HT + c * TOKH:h * HT + (c + 1) * TOKH] = \
                    res.results[c]["out"][(b * 2 + h) * TOKH:(b * 2 + h + 1) * TOKH]
    return full
